# revision 7
# baseline (speedup 1.0000x reference)
"""CTC loss (keras ctc_batch_cost semantics) on 8 Trainium2 NeuronCores.

Data-parallel over batch: 1024 samples -> 8 cores x 128 samples
(one sample per SBUF partition).

Device algorithm (per core): s-sweep over the 129-row extended CTC
lattice.  Each row's T=256 recursion v_t = (e_t + v_{t-1}) * p_t runs as
tensor_tensor_scan instructions along the free dim (op0=add, op1=mult);
the row coupling e_t = v^{s-1}_{t-1} + m_s * v^{s-2}_{t-1} is one
scalar_tensor_tensor (odd rows) or a shifted view (even rows).
Numerics: probability domain with per-(b,t) pre-scale
g = subset_max(y_pred) * exp(-RHAT) and per-chunk (TC=64) max renorm;
loss = -(log(v[S-1]+v[S-2]) + sum log renorm + sum log g).
"""
from contextlib import ExitStack

import numpy as np
import ml_dtypes

import concourse.bass as bass
import concourse.tile as tile
from concourse import bacc, mybir
from concourse.bass_utils import run_bass_kernel_spmd

F32 = mybir.dt.float32
BF16 = mybir.dt.bfloat16
AF = mybir.ActivationFunctionType
ALU = mybir.AluOpType

B, T, C, L = 1024, 256, 128, 64
S = 2 * L + 1          # 129 extended states
BLANK = C - 1
EPS = 1e-7
RHAT = 0.6             # per-step prob boost exp(RHAT) keeps chunk decay centered
TC = 128               # scan chunk length (fp32-range safe at RHAT=0.6)
NCH = T // TC
W = T + 1              # Treg slot width: col0 = v_{-1}, col 1+t = v_t
SLOTS = S + 2          # 2 permanent zero rows + 129 state rows
PB = 128               # samples per core
NCORES = 8


def _host_prep(y_true_shard: np.ndarray, y_pred_shard: np.ndarray):
    yt = y_true_shard.astype(np.int64)
    yp = y_pred_shard.astype(np.float32)
    p_blank = yp[:, :, BLANK] + EPS
    p_lab = np.take_along_axis(yp, yt[:, None, :], axis=2) + EPS
    sub = yp[:, :, 0:C:8].max(axis=2)
    g0 = np.maximum(sub, p_blank)
    ginv = (np.exp(RHAT) / g0).astype(np.float32)
    slg = (np.log(g0.astype(np.float64)).sum(axis=1) - RHAT * T).astype(np.float32)

    ptil = np.empty((PB, 65 * T), np.float32)
    for j in range(L):
        ptil[:, j * T:(j + 1) * T] = p_lab[:, :, j] * ginv
    ptil[:, L * T:] = p_blank * ginv
    m01 = np.ones((PB, L), np.float32)
    m01[:, 1:] = (yt[:, 1:] != yt[:, :-1]).astype(np.float32)
    m01[:, 0] = 0.0
    return {
        "ptil": ptil.astype(ml_dtypes.bfloat16),
        "m01": m01,
        "slg": slg[:, None],
    }


def _emit(ctx: ExitStack, tc: tile.TileContext, outs, ins):
    nc = tc.nc
    ptil_in, m01_in = ins
    (loss_out,) = outs

    persist = ctx.enter_context(tc.tile_pool(name="persist", bufs=1))
    scratch = ctx.enter_context(tc.tile_pool(name="scratch", bufs=2))
    cpool = ctx.enter_context(tc.tile_pool(name="cbuf", bufs=4))

    ptil = persist.tile([PB, 65 * T], BF16)
    for pc in range(5):
        lo, hi = pc * 13 * T, (pc + 1) * 13 * T
        nc.sync.dma_start(ptil[:, lo:hi], ptil_in[:, lo:hi])
    m01 = persist.tile([PB, L], F32)
    nc.sync.dma_start(m01[:], m01_in[:])

    treg_t = persist.tile([PB, SLOTS * W], F32)
    nc.gpsimd.memset(treg_t[:], 0.0)
    nc.vector.memset(treg_t[:, 2 * W:2 * W + 1], 1.0)   # row 0 init
    nc.vector.memset(treg_t[:, 3 * W:3 * W + 1], 1.0)   # row 1 init
    # raw output: col 0 = lsum, cols 1..NCH-1 = chunk renorm maxes
    raw = persist.tile([PB, NCH], F32)

    def sb(s):  # slot base col
        return (s + 2) * W

    for k in range(NCH):
        t0 = k * TC
        if k > 0:
            start = 2 * W + t0
            bcols = treg_t[:, start:start + (S - 1) * W + 1:W]
            nc.vector.tensor_reduce(raw[:, k:k + 1], bcols,
                                    axis=mybir.AxisListType.X, op=ALU.max)
            rinv = scratch.tile([PB, 1], F32, tag="rinv")
            nc.vector.reciprocal(rinv[:], raw[:, k:k + 1])
            nc.vector.tensor_scalar_mul(bcols, bcols, rinv[:])
        for s in range(S):
            base = sb(s)
            if s % 2 == 1:
                j = (s - 1) // 2
                c = cpool.tile([PB, TC], F32, tag="c")
                nc.vector.scalar_tensor_tensor(
                    c[:],
                    treg_t[:, sb(s - 2) + t0: sb(s - 2) + t0 + TC],
                    m01[:, j:j + 1],
                    treg_t[:, sb(s - 1) + t0: sb(s - 1) + t0 + TC],
                    op0=ALU.mult, op1=ALU.add,
                )
                d0 = c[:]
                blk = j
            else:
                d0 = treg_t[:, sb(s - 1) + t0: sb(s - 1) + t0 + TC]
                blk = L
            nc.vector.tensor_tensor_scan(
                treg_t[:, base + 1 + t0: base + 1 + t0 + TC],
                d0,
                ptil[:, blk * T + t0: blk * T + t0 + TC],
                treg_t[:, base + t0: base + t0 + 1],
                op0=ALU.add, op1=ALU.mult,
            )

    b127 = sb(127) + T
    b128 = sb(128) + T
    nc.vector.tensor_add(raw[:, 0:1], treg_t[:, b127:b127 + 1],
                         treg_t[:, b128:b128 + 1])
    nc.sync.dma_start(loss_out[:], raw[:])


_CACHE: dict = {}


def _build():
    nc = bacc.Bacc("TRN2", target_bir_lowering=False, debug=False,
                   num_devices=NCORES)
    ptil_in = nc.dram_tensor("ptil", [PB, 65 * T], BF16, kind="ExternalInput").ap()
    m01_in = nc.dram_tensor("m01", [PB, L], F32, kind="ExternalInput").ap()
    loss_out = nc.dram_tensor("raw", [PB, NCH], F32, kind="ExternalOutput").ap()
    with tile.TileContext(nc) as tcx:
        with ExitStack() as ctx:
            _emit(ctx, tcx, [loss_out], [ptil_in, m01_in])
    nc.compile()
    return nc


def _run(in_maps, **kwargs):
    if "nc" not in _CACHE:
        _CACHE["nc"] = _build()
    return run_bass_kernel_spmd(_CACHE["nc"], in_maps,
                                core_ids=list(range(NCORES)), **kwargs)


def kernel(y_true: np.ndarray, y_pred: np.ndarray, **run_kwargs) -> np.ndarray:
    assert y_pred.shape == (B, T, C), y_pred.shape
    in_maps = []
    slgs = []
    for c in range(NCORES):
        sl = slice(c * PB, (c + 1) * PB)
        prep = _host_prep(y_true[sl], y_pred[sl])
        slgs.append(prep["slg"])
        in_maps.append({"ptil": prep["ptil"], "m01": prep["m01"]})
    res = _run(in_maps, **run_kwargs)
    raw = np.concatenate([res.results[c]["raw"] for c in range(NCORES)], axis=0)
    slg = np.concatenate(slgs, axis=0)
    val = np.log(raw[:, 0].astype(np.float64))
    val += np.log(raw[:, 1:].astype(np.float64)).sum(axis=1)
    loss = -(val + slg[:, 0].astype(np.float64))
    if run_kwargs:
        kernel.last_results = res  # expose trace info to test harness
    return loss[:, None].astype(np.float32)


# revision 10
# speedup vs baseline: 1.0389x; 1.0389x over previous
"""CTC loss (keras ctc_batch_cost semantics) on 8 Trainium2 NeuronCores.

Data-parallel over batch: 1024 samples -> 8 cores x 128 samples
(one sample per SBUF partition).

Device algorithm (per core): s-sweep over the 129-row extended CTC
lattice.  Each row's T=256 recursion v_t = (e_t + v_{t-1}) * p_t runs as
tensor_tensor_scan instructions along the free dim (op0=add, op1=mult);
the row coupling e_t = v^{s-1}_{t-1} + m_s * v^{s-2}_{t-1} is one
scalar_tensor_tensor (odd rows) or a shifted view (even rows).
Numerics: probability domain with per-(b,t) pre-scale
g = subset_max(y_pred) * exp(-RHAT) and per-chunk (TC=64) max renorm;
loss = -(log(v[S-1]+v[S-2]) + sum log renorm + sum log g).
"""
from contextlib import ExitStack

import numpy as np
import ml_dtypes

import concourse.bass as bass
import concourse.tile as tile
from concourse import bacc, mybir
from concourse.bass_utils import run_bass_kernel_spmd

F32 = mybir.dt.float32
BF16 = mybir.dt.bfloat16
AF = mybir.ActivationFunctionType
ALU = mybir.AluOpType

B, T, C, L = 1024, 256, 128, 64
S = 2 * L + 1          # 129 extended states
BLANK = C - 1
EPS = 1e-7
RHAT = 0.6             # per-step prob boost exp(RHAT) keeps chunk decay centered
TC = 128               # scan chunk length (fp32-range safe at RHAT=0.6)
NCH = T // TC
W = T + 1              # Treg slot width: col0 = v_{-1}, col 1+t = v_t
SLOTS = S + 2          # 2 permanent zero rows + 129 state rows
PB = 128               # samples per core
NCORES = 8


def _host_prep(y_true_shard: np.ndarray, y_pred_shard: np.ndarray):
    yt = y_true_shard.astype(np.int64)
    yp = y_pred_shard.astype(np.float32)
    p_blank = yp[:, :, BLANK] + EPS
    p_lab = np.take_along_axis(yp, yt[:, None, :], axis=2) + EPS
    sub = yp[:, :, 0:C:8].max(axis=2)
    g0 = np.maximum(sub, p_blank)
    ginv = (np.exp(RHAT) / g0).astype(np.float32)
    slg = (np.log(g0.astype(np.float64)).sum(axis=1) - RHAT * T).astype(np.float32)

    ptil = np.empty((PB, 65 * T), np.float32)
    for j in range(L):
        ptil[:, j * T:(j + 1) * T] = p_lab[:, :, j] * ginv
    ptil[:, L * T:] = p_blank * ginv
    m01 = np.ones((PB, L), np.float32)
    m01[:, 1:] = (yt[:, 1:] != yt[:, :-1]).astype(np.float32)
    m01[:, 0] = 0.0
    return {
        "ptil": ptil.astype(ml_dtypes.bfloat16),
        "m01": m01,
        "slg": slg[:, None],
    }


def _emit(ctx: ExitStack, tc: tile.TileContext, outs, ins, outs_extra=None):
    nc = tc.nc
    ptil_in, m01_in = ins
    (loss_out,) = outs

    persist = ctx.enter_context(tc.tile_pool(name="persist", bufs=1))
    scratch = ctx.enter_context(tc.tile_pool(name="scratch", bufs=2))
    cpool = ctx.enter_context(tc.tile_pool(name="cbuf", bufs=4))

    ptil = persist.tile([PB, 65 * T], BF16)
    for pc in range(5):
        lo, hi = pc * 13 * T, (pc + 1) * 13 * T
        nc.sync.dma_start(ptil[:, lo:hi], ptil_in[:, lo:hi])
    m01 = persist.tile([PB, L], F32)
    nc.sync.dma_start(m01[:], m01_in[:])

    treg_t = persist.tile([PB, SLOTS * W], F32)
    nc.gpsimd.memset(treg_t[:], 0.0)
    # raw output: col 0 = lsum, cols 1..NCH-1 = chunk renorm maxes
    raw = persist.tile([PB, NCH], F32)

    def sb(s):  # slot base col
        return (s + 2) * W

    for k in range(NCH):
        t0 = k * TC
        if k > 0:
            start = 2 * W + t0
            bcols = treg_t[:, start:start + (S - 1) * W + 1:W]
            nc.vector.tensor_reduce(raw[:, k:k + 1], bcols,
                                    axis=mybir.AxisListType.X, op=ALU.max)
            rinv = scratch.tile([PB, 1], F32, tag="rinv")
            nc.vector.reciprocal(rinv[:], raw[:, k:k + 1])
            nc.vector.tensor_scalar_mul(bcols, bcols, rinv[:])
        for s in range(S):
            base = sb(s)
            if s % 2 == 1:
                j = (s - 1) // 2
                c = cpool.tile([PB, TC], F32, tag="c")
                nc.vector.scalar_tensor_tensor(
                    c[:],
                    treg_t[:, sb(s - 2) + t0: sb(s - 2) + t0 + TC],
                    m01[:, j:j + 1],
                    treg_t[:, sb(s - 1) + t0: sb(s - 1) + t0 + TC],
                    op0=ALU.mult, op1=ALU.add,
                )
                d0 = c[:]
                blk = j
            else:
                d0 = treg_t[:, sb(s - 1) + t0: sb(s - 1) + t0 + TC]
                blk = L
            # chunk 0: initial is an immediate (1 for rows 0/1, else 0) —
            # the col-0 cells must stay 0 because rows s+1, s+2 read them
            # as the t=0 coupling values. For k>0 the boundary cell serves
            # both roles consistently.
            if k == 0:
                initial = 1.0 if s <= 1 else 0.0
            else:
                initial = treg_t[:, base + t0: base + t0 + 1]
            nc.vector.tensor_tensor_scan(
                treg_t[:, base + 1 + t0: base + 1 + t0 + TC],
                d0,
                ptil[:, blk * T + t0: blk * T + t0 + TC],
                initial,
                op0=ALU.add, op1=ALU.mult,
            )

    b127 = sb(127) + T
    b128 = sb(128) + T
    nc.vector.tensor_add(raw[:, 0:1], treg_t[:, b127:b127 + 1],
                         treg_t[:, b128:b128 + 1])
    nc.sync.dma_start(loss_out[:], raw[:])
    if outs_extra is not None:
        nc.sync.dma_start(outs_extra, treg_t[:])


_CACHE: dict = {}


def _build(dump=False):
    nc = bacc.Bacc("TRN2", target_bir_lowering=False, debug=False,
                   num_devices=NCORES)
    ptil_in = nc.dram_tensor("ptil", [PB, 65 * T], BF16, kind="ExternalInput").ap()
    m01_in = nc.dram_tensor("m01", [PB, L], F32, kind="ExternalInput").ap()
    loss_out = nc.dram_tensor("raw", [PB, NCH], F32, kind="ExternalOutput").ap()
    extra = nc.dram_tensor("tdump", [PB, SLOTS * W], F32,
                           kind="ExternalOutput").ap() if dump else None
    with tile.TileContext(nc) as tcx:
        with ExitStack() as ctx:
            _emit(ctx, tcx, [loss_out], [ptil_in, m01_in], outs_extra=extra)
    nc.compile()
    return nc


def _run(in_maps, **kwargs):
    if "nc" not in _CACHE:
        _CACHE["nc"] = _build()
    return run_bass_kernel_spmd(_CACHE["nc"], in_maps,
                                core_ids=list(range(NCORES)), **kwargs)


def kernel(y_true: np.ndarray, y_pred: np.ndarray, **run_kwargs) -> np.ndarray:
    assert y_pred.shape == (B, T, C), y_pred.shape
    in_maps = []
    slgs = []
    for c in range(NCORES):
        sl = slice(c * PB, (c + 1) * PB)
        prep = _host_prep(y_true[sl], y_pred[sl])
        slgs.append(prep["slg"])
        in_maps.append({"ptil": prep["ptil"], "m01": prep["m01"]})
    res = _run(in_maps, **run_kwargs)
    raw = np.concatenate([res.results[c]["raw"] for c in range(NCORES)], axis=0)
    slg = np.concatenate(slgs, axis=0)
    val = np.log(raw[:, 0].astype(np.float64))
    val += np.log(raw[:, 1:].astype(np.float64)).sum(axis=1)
    loss = -(val + slg[:, 0].astype(np.float64))
    if run_kwargs:
        kernel.last_results = res  # expose trace info to test harness
    return loss[:, None].astype(np.float32)
